# revision 14
# baseline (speedup 1.0000x reference)
"""Fused attention kernel for TRN2, SPMD across 8 NeuronCores.

Problem: out = softmax(mask ? (Q Wq^T + bq)(K Wk^T + bk)^T / sqrt(D) : -1e9)
               @ (V Wv^T + bv)
with B=4, L=2048, E=D=1024.

Sharding: core c handles batch b=c//2, query-half h=c%2 (1024 query rows).
No collectives needed; K/V rows for the batch are fully loaded per core.

Algebra (per core; Xq = Q-shard (1024,E), Xk = K[b] (2048,E), Xv = V[b]):
  scores = (Xq @ Wqk) @ Xk^T + 1 (x) w^T          Wqk = Wq^T Wk
                                                  w   = Xk @ (Wk^T bq)
  (q.bk and bq.bk terms are per-query-row constants and cancel in softmax;
  the 1/sqrt(D) scale is applied at the Exp activation, keeping tT in fp8
  normal range)
  p   = exp(s/32) * mask        (unnormalized; softmax denom deferred)
  out = ((p @ Xv) @ Wv^T) * (1/sum(p)) + 1 (x) bv

Phase dtypes: phase 1 (Q proj) bf16; phase 2 (scores) fp8 e4m3 with
DoubleRow perf mode (2 contraction subtiles per matmul); phases 4/5 bf16.

Software pipeline: scores+softmax of pair k+1 are emitted before the
AV/out-proj (back) of pair k, so the ACT exp + DVE mask/denom chain of
k+1 hides under back(k)'s ~20us of PE work.
"""
from contextlib import ExitStack

import numpy as np

import concourse.bacc as bacc
import concourse.tile as tile
from concourse import mybir
from concourse.bass_utils import run_bass_kernel_spmd
from concourse.masks import make_identity

F32 = mybir.dt.float32
BF16 = mybir.dt.bfloat16
FP8 = mybir.dt.float8e4
AF = mybir.ActivationFunctionType
ALU = mybir.AluOpType
DR = mybir.MatmulPerfMode.DoubleRow

B, L, E, D = 4, 2048, 1024, 1024
LS = 1024          # query rows per core
J = 2048           # key rows per core
P = 128
NCORES = 8
SCALE = 1.0 / 32.0  # 1/sqrt(D), applied at the Exp activation

EC = E // P        # 8 chunks of 128 along E/D dims
JC = J // P        # 16 chunks along J
LT = LS // P       # 8 query tiles per core
NP = LT // 2       # 4 query-tile pairs


def _transpose_chunks(nc, ps_tr, src, dst_fn, nblk, ident, psdt, lbl,
                      dve_every=4):
    """Transpose nblk [P,P] blocks of src (groups of 4 share a psum bank).

    src: AP [P, nblk*P]; dst_fn(i) -> destination AP [P, P] for block i.
    1 in dve_every evictions go to DVE, the rest to ACT.
    """
    for t0 in range(0, nblk, 4):
        ps = ps_tr.tile([P, 512], psdt, name=f"pstr_{lbl}", tag="tr")
        for k in range(4):
            nc.tensor.transpose(
                ps[:, k * P:(k + 1) * P],
                src[:, (t0 + k) * P:(t0 + k + 1) * P],
                ident[:],
            )
        for k in range(4):
            dst = dst_fn(t0 + k)
            srcp = ps[:, k * P:(k + 1) * P]
            if (t0 // 4 + k) % dve_every == 0:
                nc.vector.tensor_copy(dst, srcp)
            else:
                nc.scalar.activation(out=dst, in_=srcp, func=AF.Copy)


def _build():
    nc = bacc.Bacc(None, target_bir_lowering=False)

    Xq_e = nc.declare_dram_parameter("XqT", [E, LS], BF16, isOutput=False)
    Xk_e = nc.declare_dram_parameter("XkT", [E, J], FP8, isOutput=False)
    Xv_e = nc.declare_dram_parameter("Xv", [J, E], BF16, isOutput=False)
    Mk_e = nc.declare_dram_parameter("mask", [LS, J], FP8, isOutput=False)
    Wqk_e = nc.declare_dram_parameter("Wqk", [E, E], BF16, isOutput=False)
    kb_e = nc.declare_dram_parameter("kb", [E], F32, isOutput=False)
    Wv_e = nc.declare_dram_parameter("WvT", [E, D], BF16, isOutput=False)
    bv_e = nc.declare_dram_parameter("bv", [D], F32, isOutput=False)
    out_e = nc.declare_dram_parameter("out", [LS, D], F32, isOutput=True)

    # chunked DRAM views: [p, chunk, free]
    XqT_d = Xq_e.ap().rearrange("(c p) l -> p c l", p=P)
    XkT_d = Xk_e.ap().rearrange("(c p) j -> p c j", p=P)
    Xv_d = Xv_e.ap().rearrange("(c p) e -> p c e", p=P)
    Wqk_d = Wqk_e.ap().rearrange("(c p) e -> p c e", p=P)
    kb_d = kb_e.ap().rearrange("(c p) -> p c", p=P)
    WvT_d = Wv_e.ap().rearrange("(c p) d -> p c d", p=P)
    Mk_d = Mk_e.ap().rearrange("(c p) j -> p c j", p=P)
    out_d = out_e.ap().rearrange("(c p) d -> p c d", p=P)

    with tile.TileContext(nc) as tc, ExitStack() as long_pools:
        lp_pool = lambda name: long_pools.enter_context(
            tc.tile_pool(name=name, bufs=1))
        with (
            tc.tile_pool(name="ps_s", bufs=2, space="PSUM") as ps_s,
            tc.tile_pool(name="ps_mm", bufs=2, space="PSUM") as ps_mm,
            tc.tile_pool(name="ps_tr", bufs=3, space="PSUM") as ps_tr,
        ):
            # ---- constants ----
            consts = lp_pool("consts")
            ident_f = consts.tile([P, P], F32, name="ident_f")
            make_identity(nc, ident_f[:])
            ident_b = consts.tile([P, P], BF16, name="ident_b")
            nc.vector.tensor_copy(ident_b[:], ident_f[:])

            bvb_sb = consts.tile([P, D], F32, name="bvb_sb")
            kb_sb = consts.tile([P, EC], F32, name="kb_sb")

            tT_sb = lp_pool("tT_p").tile([P, EC, LS], FP8, name="tT_sb")
            XkT_sb = lp_pool("XkT_p").tile([P, EC, J], FP8, name="XkT_sb")
            mask_sb = lp_pool("mask_p").tile([P, LT, J], FP8, name="mask_sb")

            # PE warmup: no-DMA transposes fill the initial DMA-latency
            # window and bring the PE out of its cold p-state before the
            # first real matmuls
            for wu in range(12):
                ps = ps_tr.tile([P, 512], F32, name="pswu", tag="tr")
                for k in range(4):
                    nc.tensor.transpose(ps[:, k * P:(k + 1) * P],
                                        ident_f[:], ident_f[:])

            # ===== stage A+B: Wqk ; kb ; XqT ; phase 1 ; XkT =====
            with (
                tc.tile_pool(name="wqk_pool", bufs=1) as wqk_pool,
                tc.tile_pool(name="xqt_pool", bufs=1) as xqt_pool,
            ):
                wqk_sb = wqk_pool.tile([P, EC, E], BF16, name="wqk_sb")
                xqT_sb = xqt_pool.tile([P, EC, LS], BF16, name="xqT_sb")
                nc.sync.dma_start(out=kb_sb[:], in_=kb_d)
                for c in range(EC):
                    nc.sync.dma_start(out=wqk_sb[:, c, :],
                                      in_=Wqk_d[:, c, :])
                    nc.scalar.dma_start(out=xqT_sb[:, c, :],
                                        in_=XqT_d[:, c, :])
                import concourse.bass as _bass
                bv_bcast = _bass.AP(tensor=bv_e, offset=0,
                                    ap=[[0, P], [1, D]])
                nc.scalar.dma_start(out=bvb_sb[:], in_=bv_bcast)

                # ===== phase 1 interleaved with XkT + mask loads =====
                def emit_xkt(et):
                    eng = nc.sync if et % 2 == 0 else nc.scalar
                    eng.dma_start(out=XkT_sb[:, et, :], in_=XkT_d[:, et, :])

                for e2t in range(EC):
                    # phase 1: tT = (Xq @ Wqk + kb)^T  [e2, l] fp8
                    for lc in range(2):
                        ps = ps_mm.tile([P, 512], F32, name="ps1",
                                        tag="mm")
                        for e1t in range(EC):
                            nc.tensor.matmul(
                                ps[:],
                                wqk_sb[:, e1t, e2t * P:(e2t + 1) * P],
                                xqT_sb[:, e1t, lc * 512:(lc + 1) * 512],
                                start=(e1t == 0), stop=(e1t == EC - 1),
                            )
                        nc.scalar.activation(
                            out=tT_sb[:, e2t, lc * 512:(lc + 1) * 512],
                            in_=ps[:], func=AF.Identity,
                            bias=kb_sb[:, e2t:e2t + 1],
                        )
                    emit_xkt(e2t)

            def emit_stage_c():
                # ===== stage C: WvT [d, do] bf16 direct loads =====
                for dt in range(EC):
                    eng = nc.sync if dt % 2 == 0 else nc.scalar
                    eng.dma_start(out=WvT_sb[:, dt, :], in_=WvT_d[:, dt, :])

            def emit_stage_d():
                # ===== stage D: Vb = Xv natural [j, d] (bf16 from host) ====
                for jt in range(JC):
                    eng = nc.sync if jt % 2 == 0 else nc.scalar
                    eng.dma_start(out=Vb_sb[:, jt, :], in_=Xv_d[:, jt, :])

            def emit_masks(lts):
                for lt in lts:
                    eng = nc.sync if lt % 2 == 0 else nc.scalar
                    eng.dma_start(out=mask_sb[:, lt, :], in_=Mk_d[:, lt, :])

            WvT_sb = lp_pool("WvT_p").tile([P, EC, D], BF16, name="WvT_sb")
            Vb_sb = lp_pool("Vb_p").tile([P, JC, D], BF16, name="Vb_sb")

            # ===== main loop pools =====
            ppool = lp_pool("pp")
            pmpool = lp_pool("pmp")
            ptpool = lp_pool("ptp")
            dnp = lp_pool("dn")

            def emit_scores(lt):
                # phase 2 (fp8 DoubleRow) + exp -> p_sb bf16 [P, J]
                p_sb = ppool.tile([P, J], BF16, name="p_sb", tag="p",
                                  bufs=4)
                for jt4 in range(4):
                    ps = ps_s.tile([P, 512], F32, name="ps_sc", tag="s",
                                   bufs=3)
                    for e2p in range(EC // 2):
                        nc.tensor.matmul(
                            ps[:],
                            tT_sb[:, 2 * e2p:2 * e2p + 2,
                                  lt * P:(lt + 1) * P],
                            XkT_sb[:, 2 * e2p:2 * e2p + 2,
                                   jt4 * 512:(jt4 + 1) * 512],
                            start=(e2p == 0), stop=(e2p == EC // 2 - 1),
                            perf_mode=DR,
                        )
                    nc.scalar.activation(
                        out=p_sb[:, jt4 * 512:(jt4 + 1) * 512],
                        in_=ps[:], func=AF.Exp, scale=SCALE,
                    )
                return p_sb

            def emit_soft(lt, p_sb):
                # pm = p * mask (unnormalized), accumulate denom; rden
                denom = dnp.tile([P, 1], F32, name="denom", tag="dn",
                                 bufs=4)
                pm = pmpool.tile([P, J], BF16, name="pm", tag="pm", bufs=4)
                nc.vector.scalar_tensor_tensor(
                    out=pm[:], in0=p_sb[:], scalar=1.0,
                    in1=mask_sb[:, lt, :],
                    op0=ALU.mult, op1=ALU.mult, accum_out=denom[:],
                )
                rden = dnp.tile([P, 1], F32, name="rden", tag="rd",
                                bufs=4)
                nc.vector.reciprocal(out=rden[:], in_=denom[:])
                return pm, rden

            def emit_pair_front(lpair):
                lts = [2 * lpair, 2 * lpair + 1]
                p_sbs = [emit_scores(lt) for lt in lts]
                return [emit_soft(lt, p_sb)
                        for lt, p_sb in zip(lts, p_sbs)]

            def emit_tr(lpair, front):
                pT_sb = ptpool.tile([P, JC, 2 * P], BF16, name="pT_sb",
                                    tag="pt", bufs=2)
                for lh in range(2):
                    pm, _ = front[lh]
                    _transpose_chunks(
                        nc, ps_tr, pm[:],
                        lambda jt, lh=lh: pT_sb[:, jt, lh * P:(lh + 1) * P],
                        JC, ident_b, BF16, "ph",
                    )
                return pT_sb

            def emit_back(lpair, pT_sb, front):
                # phase 4: zT [d, l-pair] = Xv^T p^T  (bf16)
                # last pair: split by l-half so the tail drains sooner
                zT_sb = ztpool.tile([P, EC, 2 * P], BF16, name="zT_sb",
                                    tag="zt", bufs=2)
                halves = ([(0, 2 * P)] if lpair < NP - 1
                          else [(0, P), (P, 2 * P)])
                for h0, h1 in halves:
                    for dt in range(EC):
                        ps = ps_mm.tile([P, 512], F32, name="ps4",
                                        tag="mm")
                        for jt in range(JC):
                            nc.tensor.matmul(
                                ps[:, 0:h1 - h0],
                                Vb_sb[:, jt, dt * P:(dt + 1) * P],
                                pT_sb[:, jt, h0:h1],
                                start=(jt == 0), stop=(jt == JC - 1),
                            )
                        nc.scalar.activation(out=zT_sb[:, dt, h0:h1],
                                             in_=ps[:, 0:h1 - h0],
                                             func=AF.Copy)

                    # phase 5: out = (zT^T WvT) * rden + bv
                    for lh in range(2):
                        if not (h0 <= lh * P < h1):
                            continue
                        lt = 2 * lpair + lh
                        rden = front[lh][1]
                        o_sb = opool.tile([P, D], F32, name="o_sb", tag="o",
                                          bufs=3)
                        for doc in range(2):
                            ps = ps_mm.tile([P, 512], F32, name="ps5",
                                            tag="mm")
                            for dt in range(EC):
                                nc.tensor.matmul(
                                    ps[:],
                                    zT_sb[:, dt, lh * P:(lh + 1) * P],
                                    WvT_sb[:, dt, doc * 512:(doc + 1) * 512],
                                    start=(dt == 0), stop=(dt == EC - 1),
                                )
                            nc.vector.scalar_tensor_tensor(
                                out=o_sb[:, doc * 512:(doc + 1) * 512],
                                in0=ps[:], scalar=rden[:],
                                in1=bvb_sb[:, doc * 512:(doc + 1) * 512],
                                op0=ALU.mult, op1=ALU.add,
                            )
                        eng = nc.sync if lt % 2 == 0 else nc.scalar
                        eng.dma_start(out=out_d[:, lt, :], in_=o_sb[:])

            # ===== main software pipeline =====
            # DMA emission tracks consumption order: masks for pair 0,
            # then Vb (needed by back(0) ~t+37us), pair-1 masks, WvT
            # (needed ~t+45us), remaining masks.
            emit_masks([0, 1])
            front = emit_pair_front(0)
            emit_stage_d()
            emit_masks([2, 3])
            emit_stage_c()
            emit_masks([4, 5, 6, 7])
            ztpool = lp_pool("ztp")
            opool = lp_pool("op")
            pT = emit_tr(0, front)
            for lpair in range(NP):
                nxt = None
                if lpair < NP - 1:
                    nxt = emit_pair_front(lpair + 1)
                emit_back(lpair, pT, front)
                if nxt is not None:
                    front = nxt
                    pT = emit_tr(lpair + 1, front)

    nc.compile()
    return nc


_NC_CACHE = {}


def _get_nc():
    if "nc" not in _NC_CACHE:
        _NC_CACHE["nc"] = _build()
    return _NC_CACHE["nc"]


def _shard_inputs(Q, K, V, mask, Wq_w, Wq_b, Wk_w, Wk_b, Wv_w, Wv_b):
    import ml_dtypes
    bf16 = ml_dtypes.bfloat16
    fp8 = ml_dtypes.float8_e4m3
    f32 = np.float32
    Wq32 = np.asarray(Wq_w, f32)
    Wk32 = np.asarray(Wk_w, f32)
    # NOTE: the 1/sqrt(D) score scale is applied at the Exp activation
    # (scale=1/32), so Wqk/kb are unscaled here — keeps tT in fp8's
    # normal range (sigma ~ 0.33).
    common = {
        "Wqk": np.ascontiguousarray(
            (Wq32.T @ Wk32).astype(bf16)),
        "kb": np.ascontiguousarray(
            Wk32.T @ np.asarray(Wq_b, f32), f32),
        "WvT": np.ascontiguousarray(np.asarray(Wv_w, f32).astype(bf16).T),
        "bv": np.ascontiguousarray(Wv_b, f32),
    }
    in_maps = []
    for c in range(NCORES):
        b, h = divmod(c, 2)
        sl = slice(h * LS, (h + 1) * LS)
        in_maps.append({
            "XqT": np.ascontiguousarray(
                np.asarray(Q[b, sl, :], f32).astype(bf16).T),
            "XkT": np.ascontiguousarray(
                np.asarray(K[b], f32).astype(fp8).T),
            "Xv": np.ascontiguousarray(np.asarray(V[b], f32).astype(bf16)),
            "mask": np.ascontiguousarray(
                np.asarray(mask[b, sl, :]).astype(fp8)),
            **common,
        })
    return in_maps


def _run(inputs, trace=False):
    nc = _get_nc()
    in_maps = _shard_inputs(**inputs)
    res = run_bass_kernel_spmd(nc, in_maps, core_ids=list(range(NCORES)),
                               trace=trace)
    out = np.empty((B, L, D), np.float32)
    for c in range(NCORES):
        b, h = divmod(c, 2)
        out[b, h * LS:(h + 1) * LS, :] = res.results[c]["out"]
    return out, res


def kernel(**inputs):
    out, _ = _run(inputs, trace=False)
    return out


# revision 17
# speedup vs baseline: 1.0768x; 1.0768x over previous
"""Fused attention kernel for TRN2, SPMD across 8 NeuronCores.

Problem: out = softmax(mask ? (Q Wq^T + bq)(K Wk^T + bk)^T / sqrt(D) : -1e9)
               @ (V Wv^T + bv)
with B=4, L=2048, E=D=1024.

Sharding: core c handles batch b=c//2, query-half h=c%2 (1024 query rows).
No collectives needed; K/V rows for the batch are fully loaded per core.

Algebra (per core; Xq = Q-shard (1024,E), Xk = K[b] (2048,E), Xv = V[b]):
  scores = (Xq @ Wqk) @ Xk^T + 1 (x) w^T          Wqk = Wq^T Wk
                                                  w   = Xk @ (Wk^T bq)
  (q.bk and bq.bk terms are per-query-row constants and cancel in softmax;
  the 1/sqrt(D) scale is applied at the Exp activation, keeping tT in fp8
  normal range)
  p   = exp(s/32) * mask        (unnormalized; softmax denom deferred)
  out = ((p @ Xv) @ Wv^T) * (1/sum(p)) + 1 (x) bv

Phase dtypes: phase 1 (Q proj) bf16; phase 2 (scores) fp8 e4m3 with
DoubleRow perf mode; phases 4/5 bf16.

Scheduling notes:
 - ALL loads are issued on the SP (sync) queue in consumption order, in
   large consolidated transfers; ACT/DVE streams stay pure compute (a
   dma_start blocks the issuing engine's stream on HWDGE for ~1us).
   Output stores are issued on SP too (it is idle once loads are issued).
 - Phase 1 streams over e1-chunk pairs with a dedicated 8-bank PSUM pool
   (one [128,512] accumulation group per e2t), so the first matmuls only
   need the first wqk/xqT chunk pair (~3.5us) instead of the full 4MB.
 - Software pipeline: scores+softmax of pair k+1 are emitted before the
   AV/out-proj (back) of pair k, so the ACT exp + DVE mask/denom chain of
   k+1 hides under back(k)'s ~20us of PE work.
"""
from contextlib import ExitStack

import numpy as np

import concourse.bacc as bacc
import concourse.tile as tile
from concourse import mybir
from concourse.bass_utils import run_bass_kernel_spmd
from concourse.masks import make_identity

F32 = mybir.dt.float32
BF16 = mybir.dt.bfloat16
FP8 = mybir.dt.float8e4
AF = mybir.ActivationFunctionType
ALU = mybir.AluOpType
DR = mybir.MatmulPerfMode.DoubleRow

B, L, E, D = 4, 2048, 1024, 1024
LS = 1024          # query rows per core
J = 2048           # key rows per core
P = 128
NCORES = 8
SCALE = 1.0 / 32.0  # 1/sqrt(D), applied at the Exp activation

EC = E // P        # 8 chunks of 128 along E/D dims
JC = J // P        # 16 chunks along J
LT = LS // P       # 8 query tiles per core
NP = LT // 2       # 4 query-tile pairs


def _transpose_chunks(nc, ps_tr, src, dst_fn, nblk, ident, psdt, lbl,
                      dve_every=4):
    """Transpose nblk [P,P] blocks of src (groups of 4 share a psum bank).

    src: AP [P, nblk*P]; dst_fn(i) -> destination AP [P, P] for block i.
    1 in dve_every evictions go to DVE, the rest to ACT.
    """
    for t0 in range(0, nblk, 4):
        ps = ps_tr.tile([P, 512], psdt, name=f"pstr_{lbl}", tag="tr")
        for k in range(4):
            nc.tensor.transpose(
                ps[:, k * P:(k + 1) * P],
                src[:, (t0 + k) * P:(t0 + k + 1) * P],
                ident[:],
            )
        for k in range(4):
            dst = dst_fn(t0 + k)
            srcp = ps[:, k * P:(k + 1) * P]
            if (t0 // 4 + k) % dve_every == 0:
                nc.vector.tensor_copy(dst, srcp)
            else:
                nc.scalar.activation(out=dst, in_=srcp, func=AF.Copy)


def _build():
    nc = bacc.Bacc(None, target_bir_lowering=False)

    Xq_e = nc.declare_dram_parameter("XqT", [E, LS], BF16, isOutput=False)
    Xk_e = nc.declare_dram_parameter("XkT", [E, J], FP8, isOutput=False)
    Xv_e = nc.declare_dram_parameter("Xv", [J, E], BF16, isOutput=False)
    Mk_e = nc.declare_dram_parameter("mask", [LS, J], FP8, isOutput=False)
    Wqk_e = nc.declare_dram_parameter("Wqk", [E, E], BF16, isOutput=False)
    kb_e = nc.declare_dram_parameter("kb", [E], F32, isOutput=False)
    Wv_e = nc.declare_dram_parameter("WvT", [E, D], BF16, isOutput=False)
    bv_e = nc.declare_dram_parameter("bv", [D], F32, isOutput=False)
    out_e = nc.declare_dram_parameter("out", [LS, D], F32, isOutput=True)

    # chunked DRAM views: [p, chunk, free]
    XqT_d = Xq_e.ap().rearrange("(c p) l -> p c l", p=P)
    XkT_d = Xk_e.ap().rearrange("(c p) j -> p c j", p=P)
    Xv_d = Xv_e.ap().rearrange("(c p) e -> p c e", p=P)
    Wqk_d = Wqk_e.ap().rearrange("(c p) e -> p c e", p=P)
    kb_d = kb_e.ap().rearrange("(c p) -> p c", p=P)
    WvT_d = Wv_e.ap().rearrange("(c p) d -> p c d", p=P)
    Mk_d = Mk_e.ap().rearrange("(c p) j -> p c j", p=P)
    out_d = out_e.ap().rearrange("(c p) d -> p c d", p=P)

    with tile.TileContext(nc) as tc, ExitStack() as long_pools:
        lp_pool = lambda name: long_pools.enter_context(
            tc.tile_pool(name=name, bufs=1))
        # ---- constants ----
        consts = lp_pool("consts")
        ident_f = consts.tile([P, P], F32, name="ident_f")
        make_identity(nc, ident_f[:])
        ident_b = consts.tile([P, P], BF16, name="ident_b")
        nc.vector.tensor_copy(ident_b[:], ident_f[:])

        bvb_sb = consts.tile([P, D], F32, name="bvb_sb")
        kb_sb = consts.tile([P, EC], F32, name="kb_sb")

        tT_sb = lp_pool("tT_p").tile([P, EC, LS], FP8, name="tT_sb")
        XkT_sb = lp_pool("XkT_p").tile([P, EC, J], FP8, name="XkT_sb")
        mask_sb = lp_pool("mask_p").tile([P, LT, J], FP8, name="mask_sb")
        WvT_sb = lp_pool("WvT_p").tile([P, EC, D], BF16, name="WvT_sb")
        Vb_sb = lp_pool("Vb_p").tile([P, JC, D], BF16, name="Vb_sb")

        # ======== stage A: loads (SP queue) + phase 1 (own psum pool) =====
        with (
            tc.tile_pool(name="ps_p1", bufs=1, space="PSUM") as ps_p1,
            tc.tile_pool(name="wqk_pool", bufs=1) as wqk_pool,
            tc.tile_pool(name="xqt_pool", bufs=1) as xqt_pool,
        ):
            wqk_sb = wqk_pool.tile([P, EC, E], BF16, name="wqk_sb")
            xqT_sb = xqt_pool.tile([P, EC, LS], BF16, name="xqT_sb")
            nc.sync.dma_start(out=kb_sb[:], in_=kb_d)
            import concourse.bass as _bass
            bv_bcast = _bass.AP(tensor=bv_e, offset=0,
                                ap=[[0, P], [1, D]])
            nc.sync.dma_start(out=bvb_sb[:], in_=bv_bcast)
            # wqk/xqT in chunk pairs, interleaved, in consumption order
            for cp in range(EC // 2):
                c0 = 2 * cp
                nc.sync.dma_start(out=wqk_sb[:, c0:c0 + 2, :],
                                  in_=Wqk_d[:, c0:c0 + 2, :])
                nc.sync.dma_start(out=xqT_sb[:, c0:c0 + 2, :],
                                  in_=XqT_d[:, c0:c0 + 2, :])
            # remaining loads, in consumption order
            nc.sync.dma_start(out=XkT_sb[:, 0:4, :], in_=XkT_d[:, 0:4, :])
            nc.sync.dma_start(out=XkT_sb[:, 4:8, :], in_=XkT_d[:, 4:8, :])
            nc.sync.dma_start(out=mask_sb[:, 0:2, :], in_=Mk_d[:, 0:2, :])
            nc.sync.dma_start(out=Vb_sb[:, 0:8, :], in_=Xv_d[:, 0:8, :])
            nc.sync.dma_start(out=Vb_sb[:, 8:16, :], in_=Xv_d[:, 8:16, :])
            nc.sync.dma_start(out=WvT_sb[:, 0:4, :], in_=WvT_d[:, 0:4, :])
            nc.sync.dma_start(out=WvT_sb[:, 4:8, :], in_=WvT_d[:, 4:8, :])
            nc.sync.dma_start(out=mask_sb[:, 2:4, :], in_=Mk_d[:, 2:4, :])
            nc.sync.dma_start(out=mask_sb[:, 4:6, :], in_=Mk_d[:, 4:6, :])
            nc.sync.dma_start(out=mask_sb[:, 6:8, :], in_=Mk_d[:, 6:8, :])

            # PE warmup out of the phase-1 psum banks (f32 idents, WAW with
            # the phase-1 groups only orders them on the in-order PE)
            for wu in range(6):
                ps = ps_p1.tile([P, 512], F32, name="pswu",
                                tag=f"p1_{wu % 2}")
                for k in range(4):
                    nc.tensor.transpose(ps[:, k * P:(k + 1) * P],
                                        ident_f[:], ident_f[:])

            # phase 1, e1-chunk-pair streaming: per lc pass, one psum
            # accumulation group per e2t (8 banks); evict at pass end
            for lc in range(2):
                pss = [ps_p1.tile([P, 512], F32, name=f"ps1_{lc}_{e2t}",
                                  tag=f"p1_{e2t}")
                       for e2t in range(EC)]
                for c in range(EC):
                    for e2t in range(EC):
                        nc.tensor.matmul(
                            pss[e2t][:],
                            wqk_sb[:, c, e2t * P:(e2t + 1) * P],
                            xqT_sb[:, c, lc * 512:(lc + 1) * 512],
                            start=(c == 0), stop=(c == EC - 1),
                        )
                for e2t in range(EC):
                    nc.scalar.activation(
                        out=tT_sb[:, e2t, lc * 512:(lc + 1) * 512],
                        in_=pss[e2t][:], func=AF.Identity,
                        bias=kb_sb[:, e2t:e2t + 1],
                    )

        # ======== main pools (phase-1 psum pool is closed now) ==========
        with (
            tc.tile_pool(name="ps_s", bufs=2, space="PSUM") as ps_s,
            tc.tile_pool(name="ps_mm", bufs=2, space="PSUM") as ps_mm,
            tc.tile_pool(name="ps_tr", bufs=3, space="PSUM") as ps_tr,
        ):
            ppool = lp_pool("pp")
            pmpool = lp_pool("pmp")
            ptpool = lp_pool("ptp")
            dnp = lp_pool("dn")
            ztpool = lp_pool("ztp")
            opool = lp_pool("op")

            def emit_scores(lt):
                # phase 2 (fp8 DoubleRow) + exp -> p_sb bf16 [P, J]
                p_sb = ppool.tile([P, J], BF16, name="p_sb", tag="p",
                                  bufs=4)
                for jt4 in range(4):
                    ps = ps_s.tile([P, 512], F32, name="ps_sc", tag="s",
                                   bufs=3)
                    for e2p in range(EC // 2):
                        nc.tensor.matmul(
                            ps[:],
                            tT_sb[:, 2 * e2p:2 * e2p + 2,
                                  lt * P:(lt + 1) * P],
                            XkT_sb[:, 2 * e2p:2 * e2p + 2,
                                   jt4 * 512:(jt4 + 1) * 512],
                            start=(e2p == 0), stop=(e2p == EC // 2 - 1),
                            perf_mode=DR,
                        )
                    nc.scalar.activation(
                        out=p_sb[:, jt4 * 512:(jt4 + 1) * 512],
                        in_=ps[:], func=AF.Exp, scale=SCALE,
                    )
                return p_sb

            def emit_soft(lt, p_sb):
                # pm = p * mask (unnormalized), accumulate denom; rden
                denom = dnp.tile([P, 1], F32, name="denom", tag="dn",
                                 bufs=4)
                pm = pmpool.tile([P, J], BF16, name="pm", tag="pm", bufs=4)
                nc.vector.scalar_tensor_tensor(
                    out=pm[:], in0=p_sb[:], scalar=1.0,
                    in1=mask_sb[:, lt, :],
                    op0=ALU.mult, op1=ALU.mult, accum_out=denom[:],
                )
                rden = dnp.tile([P, 1], F32, name="rden", tag="rd",
                                bufs=4)
                nc.vector.reciprocal(out=rden[:], in_=denom[:])
                return pm, rden

            def emit_pair_front(lpair):
                lts = [2 * lpair, 2 * lpair + 1]
                p_sbs = [emit_scores(lt) for lt in lts]
                return [emit_soft(lt, p_sb)
                        for lt, p_sb in zip(lts, p_sbs)]

            def emit_tr(lpair, front):
                pT_sb = ptpool.tile([P, JC, 2 * P], BF16, name="pT_sb",
                                    tag="pt", bufs=2)
                for lh in range(2):
                    pm, _ = front[lh]
                    _transpose_chunks(
                        nc, ps_tr, pm[:],
                        lambda jt, lh=lh: pT_sb[:, jt, lh * P:(lh + 1) * P],
                        JC, ident_b, BF16, "ph",
                    )
                return pT_sb

            def emit_back(lpair, pT_sb, front):
                # phase 4: zT [d, l-pair] = Xv^T p^T  (bf16)
                # last pair: split by l-half so the tail drains sooner
                zT_sb = ztpool.tile([P, EC, 2 * P], BF16, name="zT_sb",
                                    tag="zt", bufs=2)
                halves = ([(0, 2 * P)] if lpair < NP - 1
                          else [(0, P), (P, 2 * P)])
                for h0, h1 in halves:
                    for dt in range(EC):
                        ps = ps_mm.tile([P, 512], F32, name="ps4",
                                        tag="mm")
                        for jt in range(JC):
                            nc.tensor.matmul(
                                ps[:, 0:h1 - h0],
                                Vb_sb[:, jt, dt * P:(dt + 1) * P],
                                pT_sb[:, jt, h0:h1],
                                start=(jt == 0), stop=(jt == JC - 1),
                            )
                        nc.scalar.activation(out=zT_sb[:, dt, h0:h1],
                                             in_=ps[:, 0:h1 - h0],
                                             func=AF.Copy)

                    # phase 5: out = (zT^T WvT) * rden + bv
                    for lh in range(2):
                        if not (h0 <= lh * P < h1):
                            continue
                        lt = 2 * lpair + lh
                        rden = front[lh][1]
                        o_sb = opool.tile([P, D], F32, name="o_sb", tag="o",
                                          bufs=3)
                        for doc in range(2):
                            ps = ps_mm.tile([P, 512], F32, name="ps5",
                                            tag="mm")
                            for dt in range(EC):
                                nc.tensor.matmul(
                                    ps[:],
                                    zT_sb[:, dt, lh * P:(lh + 1) * P],
                                    WvT_sb[:, dt, doc * 512:(doc + 1) * 512],
                                    start=(dt == 0), stop=(dt == EC - 1),
                                )
                            nc.vector.scalar_tensor_tensor(
                                out=o_sb[:, doc * 512:(doc + 1) * 512],
                                in0=ps[:], scalar=rden[:],
                                in1=bvb_sb[:, doc * 512:(doc + 1) * 512],
                                op0=ALU.mult, op1=ALU.add,
                            )
                        nc.sync.dma_start(out=out_d[:, lt, :], in_=o_sb[:])

            # ===== main software pipeline =====
            front = emit_pair_front(0)
            pT = emit_tr(0, front)
            for lpair in range(NP):
                nxt = None
                if lpair < NP - 1:
                    nxt = emit_pair_front(lpair + 1)
                emit_back(lpair, pT, front)
                if nxt is not None:
                    front = nxt
                    pT = emit_tr(lpair + 1, front)

    nc.compile()
    return nc


_NC_CACHE = {}


def _get_nc():
    if "nc" not in _NC_CACHE:
        _NC_CACHE["nc"] = _build()
    return _NC_CACHE["nc"]


def _shard_inputs(Q, K, V, mask, Wq_w, Wq_b, Wk_w, Wk_b, Wv_w, Wv_b):
    import ml_dtypes
    bf16 = ml_dtypes.bfloat16
    fp8 = ml_dtypes.float8_e4m3
    f32 = np.float32
    Wq32 = np.asarray(Wq_w, f32)
    Wk32 = np.asarray(Wk_w, f32)
    # NOTE: the 1/sqrt(D) score scale is applied at the Exp activation
    # (scale=1/32), so Wqk/kb are unscaled here — keeps tT in fp8's
    # normal range (sigma ~ 0.33).
    common = {
        "Wqk": np.ascontiguousarray(
            (Wq32.T @ Wk32).astype(bf16)),
        "kb": np.ascontiguousarray(
            Wk32.T @ np.asarray(Wq_b, f32), f32),
        "WvT": np.ascontiguousarray(np.asarray(Wv_w, f32).astype(bf16).T),
        "bv": np.ascontiguousarray(Wv_b, f32),
    }
    in_maps = []
    for c in range(NCORES):
        b, h = divmod(c, 2)
        sl = slice(h * LS, (h + 1) * LS)
        in_maps.append({
            "XqT": np.ascontiguousarray(
                np.asarray(Q[b, sl, :], f32).astype(bf16).T),
            "XkT": np.ascontiguousarray(
                np.asarray(K[b], f32).astype(fp8).T),
            "Xv": np.ascontiguousarray(np.asarray(V[b], f32).astype(bf16)),
            "mask": np.ascontiguousarray(
                np.asarray(mask[b, sl, :]).astype(fp8)),
            **common,
        })
    return in_maps


def _run(inputs, trace=False):
    nc = _get_nc()
    in_maps = _shard_inputs(**inputs)
    res = run_bass_kernel_spmd(nc, in_maps, core_ids=list(range(NCORES)),
                               trace=trace)
    out = np.empty((B, L, D), np.float32)
    for c in range(NCORES):
        b, h = divmod(c, 2)
        out[b, h * LS:(h + 1) * LS, :] = res.results[c]["out"]
    return out, res


def kernel(**inputs):
    out, _ = _run(inputs, trace=False)
    return out


# revision 23
# speedup vs baseline: 1.1356x; 1.0547x over previous
"""Fused attention kernel for TRN2, SPMD across 8 NeuronCores.

Problem: out = softmax(mask ? (Q Wq^T + bq)(K Wk^T + bk)^T / sqrt(D) : -1e9)
               @ (V Wv^T + bv)
with B=4, L=2048, E=D=1024.

Sharding: core c handles batch b=c//2, query-half h=c%2 (1024 query rows).
No collectives needed; K/V rows for the batch are fully loaded per core.

Algebra (per core; Xq = Q-shard (1024,E), Xk = K[b] (2048,E), Xv = V[b]):
  scores = (Xq @ Wqk) @ Xk^T + 1 (x) w^T          Wqk = Wq^T Wk
                                                  w   = Xk @ (Wk^T bq)
  (q.bk and bq.bk terms are per-query-row constants and cancel in softmax;
  the 1/sqrt(D) scale is applied at the Exp activation, keeping tT in fp8
  normal range)
  p   = exp(s/32) * mask        (unnormalized; softmax denom deferred)
  out = ((p @ Xv) @ Wv^T) * (1/sum(p)) + 1 (x) bv

Phase dtypes: phase 1 (Q proj) bf16; phase 2 (scores) fp8 e4m3 with
DoubleRow perf mode; phases 4/5 bf16.

Scheduling notes:
 - ALL loads are issued on the SP (sync) queue in consumption order, in
   large consolidated transfers; ACT/DVE streams stay pure compute (a
   dma_start blocks the issuing engine's stream on HWDGE for ~1us).
   Output stores are issued on SP too (it is idle once loads are issued).
 - Phase 1 streams over e1-chunk pairs with a dedicated 8-bank PSUM pool
   (one [128,512] accumulation group per e2t), so the first matmuls only
   need the first wqk/xqT chunk pair (~3.5us) instead of the full 4MB.
 - Software pipeline: scores+softmax of pair k+1 are emitted before the
   AV/out-proj (back) of pair k, so the ACT exp + DVE mask/denom chain of
   k+1 hides under back(k)'s ~20us of PE work.
"""
from contextlib import ExitStack

import numpy as np

import concourse.bacc as bacc
import concourse.tile as tile
from concourse import mybir
from concourse.bass_utils import run_bass_kernel_spmd
from concourse.masks import make_identity

F32 = mybir.dt.float32
BF16 = mybir.dt.bfloat16
FP8 = mybir.dt.float8e4
AF = mybir.ActivationFunctionType
ALU = mybir.AluOpType
DR = mybir.MatmulPerfMode.DoubleRow

B, L, E, D = 4, 2048, 1024, 1024
LS = 1024          # query rows per core
J = 2048           # key rows per core
P = 128
NCORES = 8
SCALE = 1.0 / 32.0  # 1/sqrt(D), applied at the Exp activation

EC = E // P        # 8 chunks of 128 along E/D dims
JC = J // P        # 16 chunks along J
LT = LS // P       # 8 query tiles per core
NP = LT // 2       # 4 query-tile pairs


def _transpose_chunks(nc, ps_tr, src, dst_fn, nblk, ident, psdt, lbl,
                      dve_every=4):
    """Transpose nblk [P,P] blocks of src (groups of 4 share a psum bank).

    src: AP [P, nblk*P]; dst_fn(i) -> destination AP [P, P] for block i.
    1 in dve_every evictions go to DVE, the rest to ACT.
    """
    for t0 in range(0, nblk, 4):
        ps = ps_tr.tile([P, 512], psdt, name=f"pstr_{lbl}", tag="tr")
        for k in range(4):
            nc.tensor.transpose(
                ps[:, k * P:(k + 1) * P],
                src[:, (t0 + k) * P:(t0 + k + 1) * P],
                ident[:],
            )
        for k in range(4):
            dst = dst_fn(t0 + k)
            srcp = ps[:, k * P:(k + 1) * P]
            if (t0 // 4 + k) % dve_every == 0:
                nc.vector.tensor_copy(dst, srcp)
            else:
                nc.scalar.activation(out=dst, in_=srcp, func=AF.Copy)


def _build():
    nc = bacc.Bacc(None, target_bir_lowering=False)

    Xq_e = nc.declare_dram_parameter("XqT", [E, LS], BF16, isOutput=False)
    Xk_e = nc.declare_dram_parameter("XkT", [E, J], FP8, isOutput=False)
    Xv_e = nc.declare_dram_parameter("Xv", [J, E], BF16, isOutput=False)
    Mk_e = nc.declare_dram_parameter("mask", [LS, J], FP8, isOutput=False)
    Wqk_e = nc.declare_dram_parameter("Wqk", [E, E], BF16, isOutput=False)
    kb_e = nc.declare_dram_parameter("kb", [E], F32, isOutput=False)
    Wv_e = nc.declare_dram_parameter("WvT", [E, D], BF16, isOutput=False)
    bv_e = nc.declare_dram_parameter("bv", [D], F32, isOutput=False)
    out_e = nc.declare_dram_parameter("out", [LS, D], F32, isOutput=True)

    # chunked DRAM views: [p, chunk, free]
    XqT_d = Xq_e.ap().rearrange("(c p) l -> p c l", p=P)
    XkT_d = Xk_e.ap().rearrange("(c p) j -> p c j", p=P)
    Xv_d = Xv_e.ap().rearrange("(c p) e -> p c e", p=P)
    Wqk_d = Wqk_e.ap().rearrange("(c p) e -> p c e", p=P)
    kb_d = kb_e.ap().rearrange("(c p) -> p c", p=P)
    WvT_d = Wv_e.ap().rearrange("(c p) d -> p c d", p=P)
    Mk_d = Mk_e.ap().rearrange("(c p) j -> p c j", p=P)
    out_d = out_e.ap().rearrange("(c p) d -> p c d", p=P)

    with tile.TileContext(nc) as tc, ExitStack() as long_pools:
        lp_pool = lambda name: long_pools.enter_context(
            tc.tile_pool(name=name, bufs=1))
        # ---- constants ----
        consts = lp_pool("consts")
        ident_f = consts.tile([P, P], F32, name="ident_f")
        make_identity(nc, ident_f[:])
        ident_b = consts.tile([P, P], BF16, name="ident_b")
        nc.vector.tensor_copy(ident_b[:], ident_f[:])

        bvb_sb = consts.tile([P, D], F32, name="bvb_sb")
        kb_sb = consts.tile([P, EC], F32, name="kb_sb")

        # tT split by l-half so scores for l<512 only depend on the lc0 pass
        tT_lc = [lp_pool(f"tT_p{lc}").tile([P, EC, 512], FP8,
                                           name=f"tT_sb{lc}")
                 for lc in range(2)]
        XkT_sb = lp_pool("XkT_p").tile([P, EC, J], FP8, name="XkT_sb")
        mask_sb = lp_pool("mask_p").tile([P, LT, J], FP8, name="mask_sb")
        WvT_sb = lp_pool("WvT_p").tile([P, EC, D], BF16, name="WvT_sb")
        Vb_sb = lp_pool("Vb_p").tile([P, JC, D], BF16, name="Vb_sb")

        ppool = lp_pool("pp")
        pmpool = lp_pool("pmp")
        ptpool = lp_pool("ptp")
        dnp = lp_pool("dn")
        ztpool = lp_pool("ztp")
        opool = lp_pool("op")

        def emit_scores(lt, ps_fn):
            # phase 2 (fp8 DoubleRow) + exp -> p_sb bf16 [P, J]
            p_sb = ppool.tile([P, J], BF16, name="p_sb", tag="p",
                              bufs=4)
            tT = tT_lc[lt // 4]
            l0 = (lt % 4) * P
            for jt4 in range(4):
                ps = ps_fn(jt4)
                for e2p in range(EC // 2):
                    nc.tensor.matmul(
                        ps[:],
                        tT[:, 2 * e2p:2 * e2p + 2, l0:l0 + P],
                        XkT_sb[:, 2 * e2p:2 * e2p + 2,
                               jt4 * 512:(jt4 + 1) * 512],
                        start=(e2p == 0), stop=(e2p == EC // 2 - 1),
                        perf_mode=DR,
                    )
                nc.scalar.activation(
                    out=p_sb[:, jt4 * 512:(jt4 + 1) * 512],
                    in_=ps[:], func=AF.Exp, scale=SCALE,
                )
            return p_sb

        def emit_soft(lt, p_sb):
            # pm = p * mask (unnormalized), accumulate denom; rden
            denom = dnp.tile([P, 1], F32, name="denom", tag="dn",
                             bufs=4)
            pm = pmpool.tile([P, J], BF16, name="pm", tag="pm", bufs=4)
            nc.vector.scalar_tensor_tensor(
                out=pm[:], in0=p_sb[:], scalar=1.0,
                in1=mask_sb[:, lt, :],
                op0=ALU.mult, op1=ALU.mult, accum_out=denom[:],
            )
            rden = dnp.tile([P, 1], F32, name="rden", tag="rd",
                            bufs=4)
            nc.vector.reciprocal(out=rden[:], in_=denom[:])
            return pm, rden

        # ======== stage A: loads (SP queue) + phase 1 (own psum pool) =====
        with (
            tc.tile_pool(name="ps_p1", bufs=1, space="PSUM") as ps_p1,
            tc.tile_pool(name="wqk_pool", bufs=1) as wqk_pool,
            tc.tile_pool(name="xqt_pool", bufs=1) as xqt_pool,
        ):
            wqk_sb = wqk_pool.tile([P, EC, E], BF16, name="wqk_sb")
            xqT_sb = xqt_pool.tile([P, EC, LS], BF16, name="xqT_sb")
            nc.sync.dma_start(out=kb_sb[:], in_=kb_d)
            # wqk/xqT in chunk pairs, interleaved, in consumption order
            for cp in range(EC // 2):
                c0 = 2 * cp
                nc.sync.dma_start(out=wqk_sb[:, c0:c0 + 2, :],
                                  in_=Wqk_d[:, c0:c0 + 2, :])
                nc.sync.dma_start(out=xqT_sb[:, c0:c0 + 2, :],
                                  in_=XqT_d[:, c0:c0 + 2, :])
            # remaining loads, in consumption order
            nc.sync.dma_start(out=XkT_sb[:, 0:4, :], in_=XkT_d[:, 0:4, :])
            nc.sync.dma_start(out=XkT_sb[:, 4:8, :], in_=XkT_d[:, 4:8, :])
            nc.sync.dma_start(out=mask_sb[:, 0:2, :], in_=Mk_d[:, 0:2, :])
            import concourse.bass as _bass
            bv_bcast = _bass.AP(tensor=bv_e, offset=0,
                                ap=[[0, P], [1, D]])
            nc.sync.dma_start(out=bvb_sb[:], in_=bv_bcast)
            nc.sync.dma_start(out=Vb_sb[:, 0:8, :], in_=Xv_d[:, 0:8, :])
            nc.sync.dma_start(out=Vb_sb[:, 8:16, :], in_=Xv_d[:, 8:16, :])
            nc.sync.dma_start(out=WvT_sb[:, 0:4, :], in_=WvT_d[:, 0:4, :])
            nc.sync.dma_start(out=WvT_sb[:, 4:8, :], in_=WvT_d[:, 4:8, :])
            nc.sync.dma_start(out=mask_sb[:, 2:4, :], in_=Mk_d[:, 2:4, :])
            nc.sync.dma_start(out=mask_sb[:, 4:6, :], in_=Mk_d[:, 4:6, :])
            nc.sync.dma_start(out=mask_sb[:, 6:8, :], in_=Mk_d[:, 6:8, :])

            # PE warmup out of the phase-1 psum banks (f32 idents, WAW with
            # the phase-1 groups only orders them on the in-order PE)
            for wu in range(7):
                ps = ps_p1.tile([P, 512], F32, name="pswu",
                                tag=f"p1_{wu % 2}")
                for k in range(4):
                    nc.tensor.transpose(ps[:, k * P:(k + 1) * P],
                                        ident_f[:], ident_f[:])

            def emit_p1_pass(lc):
                # phase 1, e1-chunk streaming: one psum accumulation group
                # per e2t (8 banks); evictions (ACT/DVE alternating) at the
                # pass end
                pss = [ps_p1.tile([P, 512], F32, name=f"ps1_{lc}_{e2t}",
                                  tag=f"p1_{e2t}")
                       for e2t in range(EC)]
                for c in range(EC):
                    for e2t in range(EC):
                        nc.tensor.matmul(
                            pss[e2t][:],
                            wqk_sb[:, c, e2t * P:(e2t + 1) * P],
                            xqT_sb[:, c, lc * 512:(lc + 1) * 512],
                            start=(c == 0), stop=(c == EC - 1),
                        )
                for e2t in range(EC):
                    if e2t % 2 == 0:
                        nc.scalar.activation(
                            out=tT_lc[lc][:, e2t, :],
                            in_=pss[e2t][:], func=AF.Identity,
                            bias=kb_sb[:, e2t:e2t + 1],
                        )
                    else:
                        nc.vector.tensor_scalar(
                            out=tT_lc[lc][:, e2t, :],
                            in0=pss[e2t][:],
                            scalar1=kb_sb[:, e2t:e2t + 1],
                            scalar2=None, op0=ALU.add,
                        )

            emit_p1_pass(0)
            # pair-0 scores+softmax run during the lc1 pass; their psums
            # reuse the phase-1 banks via matching tags (WAW-ordered)
            front0 = []
            for lt in (0, 1):
                p_sb = emit_scores(
                    lt, lambda jt4, lt=lt: ps_p1.tile(
                        [P, 512], F32, name="ps_sc0",
                        tag=f"p1_{jt4 + 4 * (lt % 2)}"))
                front0.append(emit_soft(lt, p_sb))
            emit_p1_pass(1)

        # ======== main pools (phase-1 psum pool is closed now) ==========
        with (
            tc.tile_pool(name="ps_s", bufs=2, space="PSUM") as ps_s,
            tc.tile_pool(name="ps_mm", bufs=2, space="PSUM") as ps_mm,
            tc.tile_pool(name="ps_tr", bufs=3, space="PSUM") as ps_tr,
        ):
            def emit_pair_front(lpair):
                lts = [2 * lpair, 2 * lpair + 1]
                p_sbs = [emit_scores(
                    lt, lambda jt4: ps_s.tile([P, 512], F32, name="ps_sc",
                                              tag="s", bufs=3))
                    for lt in lts]
                return [emit_soft(lt, p_sb)
                        for lt, p_sb in zip(lts, p_sbs)]

            def emit_tr(lpair, front):
                pT_sb = ptpool.tile([P, JC, 2 * P], BF16, name="pT_sb",
                                    tag="pt", bufs=2)
                for lh in range(2):
                    pm, _ = front[lh]
                    _transpose_chunks(
                        nc, ps_tr, pm[:],
                        lambda jt, lh=lh: pT_sb[:, jt, lh * P:(lh + 1) * P],
                        JC, ident_b, BF16, "ph",
                    )
                return pT_sb

            def emit_back(lpair, pT_sb, front):
                # phase 4: zT [d, l-pair] = Xv^T p^T  (bf16)
                # last pair: split by l-half so the tail drains sooner
                zT_sb = ztpool.tile([P, EC, 2 * P], BF16, name="zT_sb",
                                    tag="zt", bufs=2)
                halves = ([(0, 2 * P)] if lpair < NP - 1
                          else [(0, P), (P, 2 * P)])
                for h0, h1 in halves:
                    for dt in range(EC):
                        ps = ps_mm.tile([P, 512], F32, name="ps4",
                                        tag="mm")
                        for jt in range(JC):
                            nc.tensor.matmul(
                                ps[:, 0:h1 - h0],
                                Vb_sb[:, jt, dt * P:(dt + 1) * P],
                                pT_sb[:, jt, h0:h1],
                                start=(jt == 0), stop=(jt == JC - 1),
                            )
                        nc.scalar.activation(out=zT_sb[:, dt, h0:h1],
                                             in_=ps[:, 0:h1 - h0],
                                             func=AF.Copy)

                    # phase 5: out = (zT^T WvT) * rden + bv
                    for lh in range(2):
                        if not (h0 <= lh * P < h1):
                            continue
                        lt = 2 * lpair + lh
                        rden = front[lh][1]
                        o_sb = opool.tile([P, D], F32, name="o_sb", tag="o",
                                          bufs=3)
                        for doc in range(2):
                            ps = ps_mm.tile([P, 512], F32, name="ps5",
                                            tag="mm")
                            for dt in range(EC):
                                nc.tensor.matmul(
                                    ps[:],
                                    zT_sb[:, dt, lh * P:(lh + 1) * P],
                                    WvT_sb[:, dt, doc * 512:(doc + 1) * 512],
                                    start=(dt == 0), stop=(dt == EC - 1),
                                )
                            nc.vector.scalar_tensor_tensor(
                                out=o_sb[:, doc * 512:(doc + 1) * 512],
                                in0=ps[:], scalar=rden[:],
                                in1=bvb_sb[:, doc * 512:(doc + 1) * 512],
                                op0=ALU.mult, op1=ALU.add,
                            )
                            nc.sync.dma_start(
                                out=out_d[:, lt, doc * 512:(doc + 1) * 512],
                                in_=o_sb[:, doc * 512:(doc + 1) * 512])

            # ===== main software pipeline =====
            front = front0
            pT = emit_tr(0, front)
            for lpair in range(NP):
                nxt = None
                if lpair < NP - 1:
                    nxt = emit_pair_front(lpair + 1)
                emit_back(lpair, pT, front)
                if nxt is not None:
                    front = nxt
                    pT = emit_tr(lpair + 1, front)

    nc.compile()
    return nc


_NC_CACHE = {}


def _get_nc():
    if "nc" not in _NC_CACHE:
        _NC_CACHE["nc"] = _build()
    return _NC_CACHE["nc"]


def _shard_inputs(Q, K, V, mask, Wq_w, Wq_b, Wk_w, Wk_b, Wv_w, Wv_b):
    import ml_dtypes
    bf16 = ml_dtypes.bfloat16
    fp8 = ml_dtypes.float8_e4m3
    f32 = np.float32
    Wq32 = np.asarray(Wq_w, f32)
    Wk32 = np.asarray(Wk_w, f32)
    # NOTE: the 1/sqrt(D) score scale is applied at the Exp activation
    # (scale=1/32), so Wqk/kb are unscaled here — keeps tT in fp8's
    # normal range (sigma ~ 0.33).
    common = {
        "Wqk": np.ascontiguousarray(
            (Wq32.T @ Wk32).astype(bf16)),
        "kb": np.ascontiguousarray(
            Wk32.T @ np.asarray(Wq_b, f32), f32),
        "WvT": np.ascontiguousarray(np.asarray(Wv_w, f32).astype(bf16).T),
        "bv": np.ascontiguousarray(Wv_b, f32),
    }
    in_maps = []
    for c in range(NCORES):
        b, h = divmod(c, 2)
        sl = slice(h * LS, (h + 1) * LS)
        in_maps.append({
            "XqT": np.ascontiguousarray(
                np.asarray(Q[b, sl, :], f32).astype(bf16).T),
            "XkT": np.ascontiguousarray(
                np.asarray(K[b], f32).astype(fp8).T),
            "Xv": np.ascontiguousarray(np.asarray(V[b], f32).astype(bf16)),
            "mask": np.ascontiguousarray(
                np.asarray(mask[b, sl, :]).astype(fp8)),
            **common,
        })
    return in_maps


def _run(inputs, trace=False):
    nc = _get_nc()
    in_maps = _shard_inputs(**inputs)
    res = run_bass_kernel_spmd(nc, in_maps, core_ids=list(range(NCORES)),
                               trace=trace)
    out = np.empty((B, L, D), np.float32)
    for c in range(NCORES):
        b, h = divmod(c, 2)
        out[b, h * LS:(h + 1) * LS, :] = res.results[c]["out"]
    return out, res


def kernel(**inputs):
    out, _ = _run(inputs, trace=False)
    return out


# revision 29
# speedup vs baseline: 1.1939x; 1.0513x over previous
"""Fused attention kernel for TRN2, SPMD across 8 NeuronCores.

Problem: out = softmax(mask ? (Q Wq^T + bq)(K Wk^T + bk)^T / sqrt(D) : -1e9)
               @ (V Wv^T + bv)
with B=4, L=2048, E=D=1024.

Sharding: core c handles batch b=c//2, query-half h=c%2 (1024 query rows).
No collectives needed; K/V rows for the batch are fully loaded per core.

Algebra (per core; Xq = Q-shard (1024,E), Xk = K[b] (2048,E), Xv = V[b]):
  scores = (Xq @ Wqk) @ Xk^T + 1 (x) w^T          Wqk = Wq^T Wk
                                                  w   = Xk @ (Wk^T bq)
  (q.bk and bq.bk terms are per-query-row constants and cancel in softmax;
  the 1/sqrt(D) scale is applied at the Exp activation, keeping tT in fp8
  normal range)
  p   = exp(s/32) * mask        (unnormalized; softmax denom deferred)
  out = ((p @ Xv) @ Wv^T) * (1/sum(p)) + 1 (x) bv

Phase dtypes: phase 1 (Q proj) bf16; phase 2 (scores) fp8 e4m3 with
DoubleRow perf mode; phases 4/5 bf16.

Scheduling notes:
 - ALL loads are issued on the SP (sync) queue in consumption order, in
   large consolidated transfers; ACT/DVE streams stay pure compute (a
   dma_start blocks the issuing engine's stream on HWDGE for ~1us).
   Output stores are issued on SP too (it is idle once loads are issued).
 - Phase 1 streams over e1-chunk pairs with a dedicated 8-bank PSUM pool
   (one [128,512] accumulation group per e2t), so the first matmuls only
   need the first wqk/xqT chunk pair (~3.5us) instead of the full 4MB.
 - Software pipeline: scores+softmax of pair k+1 are emitted before the
   AV/out-proj (back) of pair k, so the ACT exp + DVE mask/denom chain of
   k+1 hides under back(k)'s ~20us of PE work.
"""
from contextlib import ExitStack

import numpy as np

import concourse.bacc as bacc
import concourse.tile as tile
from concourse import mybir
from concourse.bass_utils import run_bass_kernel_spmd
from concourse.masks import make_identity

F32 = mybir.dt.float32
BF16 = mybir.dt.bfloat16
FP8 = mybir.dt.float8e4
AF = mybir.ActivationFunctionType
ALU = mybir.AluOpType
DR = mybir.MatmulPerfMode.DoubleRow

B, L, E, D = 4, 2048, 1024, 1024
LS = 1024          # query rows per core
J = 2048           # key rows per core
P = 128
NCORES = 8
SCALE = 1.0 / 32.0  # 1/sqrt(D), applied at the Exp activation

EC = E // P        # 8 chunks of 128 along E/D dims
JC = J // P        # 16 chunks along J
LT = LS // P       # 8 query tiles per core
NP = LT // 2       # 4 query-tile pairs


def _transpose_chunks(nc, ps_tr, src, dst_fn, nblk, ident, psdt, lbl,
                      dve_every=4):
    """Transpose nblk [P,P] blocks of src (groups of 4 share a psum bank).

    src: AP [P, nblk*P]; dst_fn(i) -> destination AP [P, P] for block i.
    1 in dve_every evictions go to DVE, the rest to ACT.
    """
    for t0 in range(0, nblk, 4):
        ps = ps_tr.tile([P, 512], psdt, name=f"pstr_{lbl}", tag="tr")
        for k in range(4):
            nc.tensor.transpose(
                ps[:, k * P:(k + 1) * P],
                src[:, (t0 + k) * P:(t0 + k + 1) * P],
                ident[:],
            )
        for k in range(4):
            dst = dst_fn(t0 + k)
            srcp = ps[:, k * P:(k + 1) * P]
            if (t0 // 4 + k) % dve_every == 0:
                nc.vector.tensor_copy(dst, srcp)
            else:
                nc.scalar.activation(out=dst, in_=srcp, func=AF.Copy)


def _build():
    nc = bacc.Bacc(None, target_bir_lowering=False)

    Xq_e = nc.declare_dram_parameter("XqT", [E, LS], BF16, isOutput=False)
    Xk_e = nc.declare_dram_parameter("XkT", [E, J], FP8, isOutput=False)
    VbH_e = nc.declare_dram_parameter("VbH", [J, E], FP8, isOutput=False)
    VbL_e = nc.declare_dram_parameter("VbL", [J, E], FP8, isOutput=False)
    Mk_e = nc.declare_dram_parameter("mask", [LS, J], FP8, isOutput=False)
    Wqk_e = nc.declare_dram_parameter("Wqk", [E, E], BF16, isOutput=False)
    kb_e = nc.declare_dram_parameter("kb", [E], F32, isOutput=False)
    WvH_e = nc.declare_dram_parameter("WvH", [E, D], FP8, isOutput=False)
    WvL_e = nc.declare_dram_parameter("WvL", [E, D], FP8, isOutput=False)
    bv_e = nc.declare_dram_parameter("bv", [D], F32, isOutput=False)
    out_e = nc.declare_dram_parameter("out", [LS, D], F32, isOutput=True)

    # chunked DRAM views: [p, chunk, free]
    XqT_d = Xq_e.ap().rearrange("(c p) l -> p c l", p=P)
    XkT_d = Xk_e.ap().rearrange("(c p) j -> p c j", p=P)
    VbH_d = VbH_e.ap().rearrange("(c p) e -> p c e", p=P)
    VbL_d = VbL_e.ap().rearrange("(c p) e -> p c e", p=P)
    Wqk_d = Wqk_e.ap().rearrange("(c p) e -> p c e", p=P)
    kb_d = kb_e.ap().rearrange("(c p) -> p c", p=P)
    WvH_d = WvH_e.ap().rearrange("(c p) d -> p c d", p=P)
    WvL_d = WvL_e.ap().rearrange("(c p) d -> p c d", p=P)
    Mk_d = Mk_e.ap().rearrange("(c p) j -> p c j", p=P)
    out_d = out_e.ap().rearrange("(c p) d -> p c d", p=P)

    with tile.TileContext(nc) as tc, ExitStack() as long_pools:
        lp_pool = lambda name: long_pools.enter_context(
            tc.tile_pool(name=name, bufs=1))
        # ---- constants ----
        consts = lp_pool("consts")
        ident_f = consts.tile([P, P], F32, name="ident_f")
        make_identity(nc, ident_f[:])
        ident_b = consts.tile([P, P], BF16, name="ident_b")
        nc.vector.tensor_copy(ident_b[:], ident_f[:])

        bvb_sb = consts.tile([P, D], F32, name="bvb_sb")
        kb_sb = consts.tile([P, EC], F32, name="kb_sb")

        # tT split by l-half so scores for l<512 only depend on the lc0 pass
        tT_lc = [lp_pool(f"tT_p{lc}").tile([P, EC, 512], FP8,
                                           name=f"tT_sb{lc}")
                 for lc in range(2)]
        XkT_sb = lp_pool("XkT_p").tile([P, EC, J], FP8, name="XkT_sb")
        mask_sb = lp_pool("mask_p").tile([P, LT, J], FP8, name="mask_sb")
        WvH_sb = lp_pool("WvH_p").tile([P, EC, D], FP8, name="WvH_sb")
        WvL_sb = lp_pool("WvL_p").tile([P, EC, D], FP8, name="WvL_sb")
        VbH_sb = lp_pool("VbH_p").tile([P, JC, D], FP8, name="VbH_sb")
        VbL_sb = lp_pool("VbL_p").tile([P, JC, D], FP8, name="VbL_sb")

        ppool = lp_pool("pp")
        pmpool = lp_pool("pmp")
        ptpool = lp_pool("ptp")
        dnp = lp_pool("dn")
        ztpool = lp_pool("ztp")
        opool = lp_pool("op")

        def emit_scores(lt, ps_fn):
            # phase 2 (fp8 DoubleRow) + exp -> p_sb bf16 [P, J]
            p_sb = ppool.tile([P, J], BF16, name="p_sb", tag="p",
                              bufs=4)
            tT = tT_lc[lt // 4]
            l0 = (lt % 4) * P
            for jt4 in range(4):
                ps = ps_fn(jt4)
                for e2p in range(EC // 2):
                    nc.tensor.matmul(
                        ps[:],
                        tT[:, 2 * e2p:2 * e2p + 2, l0:l0 + P],
                        XkT_sb[:, 2 * e2p:2 * e2p + 2,
                               jt4 * 512:(jt4 + 1) * 512],
                        start=(e2p == 0), stop=(e2p == EC // 2 - 1),
                        perf_mode=DR,
                    )
                nc.scalar.activation(
                    out=p_sb[:, jt4 * 512:(jt4 + 1) * 512],
                    in_=ps[:], func=AF.Exp, scale=SCALE,
                )
            return p_sb

        def emit_soft(lt, p_sb):
            # pm = p * mask (unnormalized), accumulate denom; the final
            # per-row scalar is 1/(4*denom): z carries 1/16, WvT carries
            # x64 -> net x4 to cancel
            denom = dnp.tile([P, 1], F32, name="denom", tag="dn",
                             bufs=4)
            pm = pmpool.tile([P, J], BF16, name="pm", tag="pm", bufs=4)
            nc.vector.scalar_tensor_tensor(
                out=pm[:], in0=p_sb[:], scalar=1.0,
                in1=mask_sb[:, lt, :],
                op0=ALU.mult, op1=ALU.mult, accum_out=denom[:],
            )
            den4 = dnp.tile([P, 1], F32, name="den4", tag="d4", bufs=4)
            nc.vector.tensor_scalar(out=den4[:], in0=denom[:],
                                    scalar1=4.0, scalar2=None,
                                    op0=ALU.mult)
            rden = dnp.tile([P, 1], F32, name="rden", tag="rd",
                            bufs=4)
            nc.vector.reciprocal(out=rden[:], in_=den4[:])
            return pm, rden

        # ======== stage A: loads (SP queue) + phase 1 (own psum pool) =====
        with (
            tc.tile_pool(name="ps_p1", bufs=1, space="PSUM") as ps_p1,
            tc.tile_pool(name="wqk_pool", bufs=1) as wqk_pool,
            tc.tile_pool(name="xqt_pool", bufs=1) as xqt_pool,
        ):
            wqk_sb = wqk_pool.tile([P, EC, E], BF16, name="wqk_sb")
            xqT_sb = xqt_pool.tile([P, EC, LS], BF16, name="xqT_sb")
            nc.sync.dma_start(out=kb_sb[:], in_=kb_d)
            # wqk/xqT in chunk pairs, interleaved, in consumption order
            for cp in range(EC // 2):
                c0 = 2 * cp
                nc.sync.dma_start(out=wqk_sb[:, c0:c0 + 2, :],
                                  in_=Wqk_d[:, c0:c0 + 2, :])
                nc.sync.dma_start(out=xqT_sb[:, c0:c0 + 2, :],
                                  in_=XqT_d[:, c0:c0 + 2, :])
            # remaining loads, in consumption order
            nc.sync.dma_start(out=XkT_sb[:, 0:4, :], in_=XkT_d[:, 0:4, :])
            nc.sync.dma_start(out=XkT_sb[:, 4:8, :], in_=XkT_d[:, 4:8, :])
            nc.sync.dma_start(out=mask_sb[:, 0:2, :], in_=Mk_d[:, 0:2, :])
            import concourse.bass as _bass
            bv_bcast = _bass.AP(tensor=bv_e, offset=0,
                                ap=[[0, P], [1, D]])
            nc.sync.dma_start(out=bvb_sb[:], in_=bv_bcast)
            nc.sync.dma_start(out=VbH_sb[:, :, :], in_=VbH_d[:, :, :])
            nc.sync.dma_start(out=VbL_sb[:, :, :], in_=VbL_d[:, :, :])
            nc.sync.dma_start(out=WvH_sb[:, :, :], in_=WvH_d[:, :, :])
            nc.sync.dma_start(out=WvL_sb[:, :, :], in_=WvL_d[:, :, :])
            nc.sync.dma_start(out=mask_sb[:, 2:4, :], in_=Mk_d[:, 2:4, :])
            nc.sync.dma_start(out=mask_sb[:, 4:6, :], in_=Mk_d[:, 4:6, :])
            nc.sync.dma_start(out=mask_sb[:, 6:8, :], in_=Mk_d[:, 6:8, :])

            # PE warmup out of the phase-1 psum banks (f32 idents, WAW with
            # the phase-1 groups only orders them on the in-order PE)
            for wu in range(7):
                ps = ps_p1.tile([P, 512], F32, name="pswu",
                                tag=f"p1_{wu % 2}")
                for k in range(4):
                    nc.tensor.transpose(ps[:, k * P:(k + 1) * P],
                                        ident_f[:], ident_f[:])

            def emit_p1_pass(lc):
                # phase 1, e1-chunk streaming: one psum accumulation group
                # per e2t (8 banks); evictions (ACT/DVE alternating) at the
                # pass end
                pss = [ps_p1.tile([P, 512], F32, name=f"ps1_{lc}_{e2t}",
                                  tag=f"p1_{e2t}")
                       for e2t in range(EC)]
                for c in range(EC):
                    for e2t in range(EC):
                        nc.tensor.matmul(
                            pss[e2t][:],
                            wqk_sb[:, c, e2t * P:(e2t + 1) * P],
                            xqT_sb[:, c, lc * 512:(lc + 1) * 512],
                            start=(c == 0), stop=(c == EC - 1),
                        )
                for e2t in range(EC):
                    if e2t % 2 == 0:
                        nc.scalar.activation(
                            out=tT_lc[lc][:, e2t, :],
                            in_=pss[e2t][:], func=AF.Identity,
                            bias=kb_sb[:, e2t:e2t + 1],
                        )
                    else:
                        nc.vector.tensor_scalar(
                            out=tT_lc[lc][:, e2t, :],
                            in0=pss[e2t][:],
                            scalar1=kb_sb[:, e2t:e2t + 1],
                            scalar2=None, op0=ALU.add,
                        )

            emit_p1_pass(0)
            # pair-0 scores+softmax run during the lc1 pass; their psums
            # reuse the phase-1 banks via matching tags (WAW-ordered)
            front0 = []
            for lt in (0, 1):
                p_sb = emit_scores(
                    lt, lambda jt4, lt=lt: ps_p1.tile(
                        [P, 512], F32, name="ps_sc0",
                        tag=f"p1_{jt4 + 4 * (lt % 2)}"))
                front0.append(emit_soft(lt, p_sb))
            emit_p1_pass(1)

        # ======== main pools (phase-1 psum pool is closed now) ==========
        with (
            tc.tile_pool(name="ps_s", bufs=2, space="PSUM") as ps_s,
            tc.tile_pool(name="ps_mm", bufs=2, space="PSUM") as ps_mm,
            tc.tile_pool(name="ps_tr", bufs=3, space="PSUM") as ps_tr,
        ):
            def emit_pair_front(lpair):
                lts = [2 * lpair, 2 * lpair + 1]
                p_sbs = [emit_scores(
                    lt, lambda jt4: ps_s.tile([P, 512], F32, name="ps_sc",
                                              tag="s", bufs=3))
                    for lt in lts]
                return [emit_soft(lt, p_sb)
                        for lt, p_sb in zip(lts, p_sbs)]

            def emit_tr(lpair, front):
                # transpose pm (bf16) and split hi/lo fp8 at eviction:
                # ACT writes hi, DVE writes lo = psum - hi
                pT_hi = ptpool.tile([P, JC, 2 * P], FP8, name="pT_hi",
                                    tag="pth", bufs=2)
                pT_lo = ptpool.tile([P, JC, 2 * P], FP8, name="pT_lo",
                                    tag="ptl", bufs=2)
                for lh in range(2):
                    pm, _ = front[lh]
                    for t0 in range(0, JC, 4):
                        ps = ps_tr.tile([P, 512], BF16, name="pstr",
                                        tag="tr")
                        for k in range(4):
                            nc.tensor.transpose(
                                ps[:, k * P:(k + 1) * P],
                                pm[:, (t0 + k) * P:(t0 + k + 1) * P],
                                ident_b[:],
                            )
                        for k in range(4):
                            jt = t0 + k
                            blk = ps[:, k * P:(k + 1) * P]
                            hi = pT_hi[:, jt, lh * P:(lh + 1) * P]
                            nc.scalar.activation(out=hi, in_=blk,
                                                 func=AF.Copy)
                            nc.vector.tensor_tensor(
                                out=pT_lo[:, jt, lh * P:(lh + 1) * P],
                                in0=blk, in1=hi, op=ALU.subtract,
                            )
                return pT_hi, pT_lo

            def emit_p4(lpair, pT, h0, h1):
                # phase 4: zT [d, l] = Xv^T p^T, fp8 3-term
                # (hi.hi + hi.lo + lo.hi); zT evicted as fp8 hi/lo at 1/16
                pT_hi, pT_lo = pT
                zT_hi = ztpool.tile([P, EC, 2 * P], FP8, name="zT_hi",
                                    tag="zth", bufs=2)
                zT_lo = ztpool.tile([P, EC, 2 * P], FP8, name="zT_lo",
                                    tag="ztl", bufs=2)
                terms = [(pT_hi, VbH_sb), (pT_hi, VbL_sb), (pT_lo, VbH_sb)]
                nmm = 3 * JC // 2
                for dt in range(EC):
                    ps = ps_mm.tile([P, 512], F32, name="ps4", tag="mm")
                    n = 0
                    for pt, vb in terms:
                        for jp in range(JC // 2):
                            nc.tensor.matmul(
                                ps[:, 0:h1 - h0],
                                vb[:, 2 * jp:2 * jp + 2,
                                   dt * P:(dt + 1) * P],
                                pt[:, 2 * jp:2 * jp + 2, h0:h1],
                                start=(n == 0), stop=(n == nmm - 1),
                                perf_mode=DR,
                            )
                            n += 1
                    hi = zT_hi[:, dt, h0:h1]
                    nc.scalar.activation(out=hi, in_=ps[:, 0:h1 - h0],
                                         func=AF.Copy, scale=1.0 / 16.0)
                    nc.vector.scalar_tensor_tensor(
                        out=zT_lo[:, dt, h0:h1], in0=ps[:, 0:h1 - h0],
                        scalar=1.0 / 16.0, in1=hi,
                        op0=ALU.mult, op1=ALU.subtract,
                    )
                return zT_hi, zT_lo

            def emit_p5(lpair, zT, front, lh):
                # phase 5: out = (zT^T WvT) * rden + bv, fp8 3-term
                zT_hi, zT_lo = zT
                lt = 2 * lpair + lh
                rden = front[lh][1]
                o_sb = opool.tile([P, D], F32, name="o_sb", tag="o",
                                  bufs=3)
                terms = [(zT_hi, WvH_sb), (zT_hi, WvL_sb), (zT_lo, WvH_sb)]
                nmm = 3 * EC // 2
                for doc in range(2):
                    ps = ps_mm.tile([P, 512], F32, name="ps5", tag="mm")
                    n = 0
                    for zt, wv in terms:
                        for dp in range(EC // 2):
                            nc.tensor.matmul(
                                ps[:],
                                zt[:, 2 * dp:2 * dp + 2,
                                   lh * P:(lh + 1) * P],
                                wv[:, 2 * dp:2 * dp + 2,
                                   doc * 512:(doc + 1) * 512],
                                start=(n == 0), stop=(n == nmm - 1),
                                perf_mode=DR,
                            )
                            n += 1
                    nc.vector.scalar_tensor_tensor(
                        out=o_sb[:, doc * 512:(doc + 1) * 512],
                        in0=ps[:], scalar=rden[:],
                        in1=bvb_sb[:, doc * 512:(doc + 1) * 512],
                        op0=ALU.mult, op1=ALU.add,
                    )
                    nc.sync.dma_start(
                        out=out_d[:, lt, doc * 512:(doc + 1) * 512],
                        in_=o_sb[:, doc * 512:(doc + 1) * 512])

            # ===== main software pipeline =====
            # PE order per iteration: scores(k+1) | p4(k) | tr(k+1) | p5(k)
            # — tr(k+1) fills the zT-eviction bubble between p4 and p5
            front = front0
            pT = emit_tr(0, front)
            for lpair in range(NP):
                nxt = None
                if lpair < NP - 1:
                    nxt = emit_pair_front(lpair + 1)
                if lpair < NP - 1:
                    zT = emit_p4(lpair, pT, 0, 2 * P)
                    pTn = emit_tr(lpair + 1, nxt)
                    emit_p5(lpair, zT, front, 0)
                    emit_p5(lpair, zT, front, 1)
                    front, pT = nxt, pTn
                else:
                    # last pair: split by l-half so the tail drains sooner
                    zT = emit_p4(lpair, pT, 0, P)
                    emit_p5(lpair, zT, front, 0)
                    zT = emit_p4(lpair, pT, P, 2 * P)
                    emit_p5(lpair, zT, front, 1)

    nc.compile()
    return nc


_NC_CACHE = {}


def _get_nc():
    if "nc" not in _NC_CACHE:
        _NC_CACHE["nc"] = _build()
    return _NC_CACHE["nc"]


def _shard_inputs(Q, K, V, mask, Wq_w, Wq_b, Wk_w, Wk_b, Wv_w, Wv_b):
    import ml_dtypes
    bf16 = ml_dtypes.bfloat16
    fp8 = ml_dtypes.float8_e4m3
    f32 = np.float32

    def hilo(x):
        hi = x.astype(fp8)
        lo = (x - hi.astype(f32)).astype(fp8)
        return (np.ascontiguousarray(hi), np.ascontiguousarray(lo))

    Wq32 = np.asarray(Wq_w, f32)
    Wk32 = np.asarray(Wk_w, f32)
    # NOTE: the 1/sqrt(D) score scale is applied at the Exp activation
    # (scale=1/32), so Wqk/kb are unscaled here — keeps tT in fp8's
    # normal range (sigma ~ 0.33). WvT is scaled x64 so its fp8 hi/lo
    # escapes the subnormal floor; z carries 1/16 — both cancelled by
    # the final 1/(4*denom) row scale.
    WvH, WvL = hilo(np.asarray(Wv_w, f32).T * 64.0)
    common = {
        "Wqk": np.ascontiguousarray(
            (Wq32.T @ Wk32).astype(bf16)),
        "kb": np.ascontiguousarray(
            Wk32.T @ np.asarray(Wq_b, f32), f32),
        "WvH": WvH, "WvL": WvL,
        "bv": np.ascontiguousarray(Wv_b, f32),
    }
    in_maps = []
    for c in range(NCORES):
        b, h = divmod(c, 2)
        sl = slice(h * LS, (h + 1) * LS)
        VbH, VbL = hilo(np.asarray(V[b], f32))
        in_maps.append({
            "XqT": np.ascontiguousarray(
                np.asarray(Q[b, sl, :], f32).astype(bf16).T),
            "XkT": np.ascontiguousarray(
                np.asarray(K[b], f32).astype(fp8).T),
            "VbH": VbH, "VbL": VbL,
            "mask": np.ascontiguousarray(
                np.asarray(mask[b, sl, :]).astype(fp8)),
            **common,
        })
    return in_maps


def _run(inputs, trace=False):
    nc = _get_nc()
    in_maps = _shard_inputs(**inputs)
    res = run_bass_kernel_spmd(nc, in_maps, core_ids=list(range(NCORES)),
                               trace=trace)
    out = np.empty((B, L, D), np.float32)
    for c in range(NCORES):
        b, h = divmod(c, 2)
        out[b, h * LS:(h + 1) * LS, :] = res.results[c]["out"]
    return out, res


def kernel(**inputs):
    out, _ = _run(inputs, trace=False)
    return out
